# revision 1
# baseline (speedup 1.0000x reference)
"""Dinov3 ViT attention kernel for Trainium2 (8 NeuronCores, data-parallel over batch).

Per core: 2 batch items. hidden_states [2*1029, 1024] in, out [2*1029, 1024] f32.

Host pre-casts hidden_states + weights to bf16 (the kernel computes in bf16
internally anyway, so this only halves DMA traffic).

Per item pipeline (PE-dense, interleaved with ACT-bound attention):
  X-prep (PE transpose to feature-major XT) ->
  V-proj chunk 0 (heads 0..7) ->
  for mo in 0..7:  # one 128-feature tile = head pair (2mo, 2mo+1)
      Q-proj(mo) + bias, K-proj(mo), RoPE(mo) on DVE,
      attention for heads 2mo, 2mo+1:
        S^T per key-tile (K=64 matmul), exp on ScalarE (scale=1/8, no max:
        |scores| < ~7), AV matmul with ones-augmented V (row 64 = softmax sums),
        5-query tail batched into one [128,45] PSUM bank + single exp per head,
        normalize via DVE reciprocal + gpsimd partition_broadcast.
      (V-proj chunk 1 emitted before mo=4)
  output projection Y = (AttnOut^T)^T Wo + bo -> DMA f32.
"""
import sys
import time

sys.path.insert(0, "/opt/trn_rl_repo")

import ml_dtypes
import numpy as np

import concourse.bacc as bacc
import concourse.mybir as mybir
import concourse.tile as tile

f32 = mybir.dt.float32
bf16 = mybir.dt.bfloat16
FP = mybir.ActivationFunctionType
ADD = mybir.AluOpType.add
MUL = mybir.AluOpType.mult

H = 1024
NH = 16
HD = 64
T = 1029
NPREF = 5
PATCH = 1024
B = 16
NCORES = 8
BPC = B // NCORES          # batch items per core
KO = H // 128              # 8 feature k-tiles
TOK = BPC * T              # tokens per core (2058)
SCALE = 1.0 / float(np.sqrt(HD))

TOK_TILES = [(i * 128, min(128, T - i * 128)) for i in range((T + 127) // 128)]
NJT = len(TOK_TILES)
QCHUNKS = [(0, 512), (512, 512)]
QTAIL = (1024, T - 1024)               # 5 queries -> batched-exp path
PROJ_CHUNKS = [(0, 343), (343, 343), (686, 343)]
NCHUNKS = [(0, 512), (512, 512)]


def build():
    nc = bacc.Bacc(None, target_bir_lowering=False)
    hs = nc.dram_tensor("hs", [H, TOK], bf16, kind="ExternalInput")  # host pre-transposed
    cos_d = nc.dram_tensor("cos", [PATCH, HD], f32, kind="ExternalInput")
    sin_d = nc.dram_tensor("sin", [PATCH, HD], f32, kind="ExternalInput")
    w_d = {wn: nc.dram_tensor(wn, [H, H], bf16, kind="ExternalInput")
           for wn in ("wq", "wk", "wv", "wo")}
    b_d = {"bq": nc.dram_tensor("bq", [H], f32, kind="ExternalInput"),
           "bv": nc.dram_tensor("bv", [H], bf16, kind="ExternalInput"),
           "bo": nc.dram_tensor("bo", [H], bf16, kind="ExternalInput")}
    ident_d = nc.dram_tensor("ident", [128, 128], bf16, kind="ExternalInput")
    out_d = nc.dram_tensor("out", [TOK, H], f32, kind="ExternalOutput")

    with tile.TileContext(nc) as tc:
        with (
            tc.tile_pool(name="const", bufs=1) as cpool,
            tc.tile_pool(name="item", bufs=1) as ipool,
            tc.tile_pool(name="ao", bufs=2) as aopool,
            tc.tile_pool(name="work", bufs=3) as wpool,
            tc.tile_pool(name="rope", bufs=2) as rpool,
            tc.tile_pool(name="attn", bufs=4) as apool,
            tc.tile_pool(name="ypool", bufs=2) as ypool,
            tc.tile_pool(name="attn2", bufs=2) as apool2,
            tc.tile_pool(name="ps_s", bufs=2, space="PSUM") as ps_s,
            tc.tile_pool(name="ps_o", bufs=1, space="PSUM") as ps_o,
            tc.tile_pool(name="ps_w", bufs=2, space="PSUM") as ps_w,
        ):
            identb = cpool.tile([128, 128], bf16)
            nc.sync.dma_start(identb[:], ident_d[:])
            ident = cpool.tile([128, 128], f32)
            nc.vector.tensor_copy(ident[:], identb[:])

            # --- X-prep: hs is already feature-major; one strided DMA per item ---
            hs_r = hs.rearrange("(o p) t -> p o t", p=128)

            def emit_xprep_full(bi, XT):
                nc.sync.dma_start(XT[:, :, :], hs_r[:, :, bi * T: bi * T + T])

            XT0 = ipool.tile([128, KO, T], bf16, tag="XT", name="XT_0")
            emit_xprep_full(0, XT0)

            # --- cos/sin -> transposed, duplicated, sign-adjusted tables ---
            cosT2 = cpool.tile([128, PATCH], bf16)
            sinT2sw = cpool.tile([128, PATCH], bf16)
            cs_all = cpool.tile([128, PATCH // 128, HD], f32, tag="cs_all")
            sn_all = cpool.tile([128, PATCH // 128, HD], f32, tag="sn_all")
            nc.sync.dma_start(cs_all[:], cos_d.rearrange("(o p) d -> p o d", p=128))
            nc.sync.dma_start(sn_all[:], sin_d.rearrange("(o p) d -> p o d", p=128))
            for i in range(PATCH // 128):
                sl = slice(i * 128, (i + 1) * 128)
                pt = ps_w.tile([128, 512], f32, tag="ps_w")
                nc.tensor.transpose(pt[:HD, :128], cs_all[:, i, :], ident[:])
                nc.vector.tensor_copy(cosT2[0:64, sl], pt[0:64, :128])
                nc.vector.tensor_copy(cosT2[64:128, sl], pt[0:64, :128])
                pt2 = ps_w.tile([128, 512], f32, tag="ps_w")
                nc.tensor.transpose(pt2[:HD, :128], sn_all[:, i, :], ident[:])
                # rows 0:32 hold +sin[32:64] (read at source partitions 32:64 of
                # q), rows 32:64 hold -sin[0:32]; duplicated for the odd head.
                nc.vector.tensor_copy(sinT2sw[0:32, sl], pt2[32:64, :128])
                nc.vector.tensor_scalar_mul(sinT2sw[32:64, sl], pt2[0:32, :128],
                                            -1.0)
                nc.vector.tensor_copy(sinT2sw[64:96, sl], pt2[32:64, :128])
                nc.vector.tensor_scalar_mul(sinT2sw[96:128, sl], pt2[0:32, :128],
                                            -1.0)

            # --- biases, weights (already bf16; single strided DMA each) ---
            bq_sb = cpool.tile([128, KO], f32)
            nc.sync.dma_start(bq_sb[:], b_d["bq"].rearrange("(o p) -> p o", p=128))
            bv_bc = cpool.tile([128, H], bf16)
            nc.sync.dma_start(bv_bc[:], b_d["bv"][None, :].to_broadcast((128, H)))
            bo_bc = cpool.tile([128, H], bf16)
            nc.sync.dma_start(bo_bc[:], b_d["bo"][None, :].to_broadcast((128, H)))

            wb = {}
            for wn in ("wq", "wv", "wk", "wo"):
                wb[wn] = cpool.tile([128, KO, H], bf16, tag=f"wb_{wn}",
                                    name=f"wb_{wn}")
            for wn in ("wq", "wv", "wk", "wo"):
                nc.sync.dma_start(
                    wb[wn][:], w_d[wn].rearrange("(o p) n -> p o n", p=128))

            # ---------------- per batch item ----------------
            def make_item(bi, XT):
                tok0 = bi * T
                QT = ipool.tile([128, KO, T], bf16, tag="QT", name=f"QT_{bi}")
                KT = ipool.tile([128, KO, T], bf16, tag="KT", name=f"KT_{bi}")
                Vst = ipool.tile([128, NJT, NH, HD + 1], bf16, tag="Vst",
                                 name=f"Vst_{bi}")
                AOT = aopool.tile([128, KO, T], bf16, tag="AOT", name=f"AOT_{bi}")

                def emit_vinit():
                    nc.vector.memset(Vst[:, :, :, HD:HD + 1], 1.0)

                def emit_vproj_t(ci, ti):
                    n0, nw = NCHUNKS[ci]
                    t0, tw = TOK_TILES[ti]
                    pm = ps_w.tile([128, 512], f32, tag="ps_w",
                                   name=f"pmv_{bi}_{ci}_{ti}")
                    for ko in range(KO):
                        nc.tensor.matmul(
                            pm[:tw, :nw],
                            XT[:, ko, t0:t0 + tw],
                            wb["wv"][:, ko, n0:n0 + nw],
                            start=(ko == 0), stop=(ko == KO - 1))
                    nc.vector.tensor_tensor(
                        Vst[:tw, ti, n0 // HD:(n0 + nw) // HD, 0:HD],
                        pm[:tw, :nw], bv_bc[:tw, n0:n0 + nw], ADD)

                def emit_qkproj_g(mo, which, ci):
                    dst, wn, bias = ((QT, "wq", True), (KT, "wk", False))[which]
                    q0, qw = PROJ_CHUNKS[ci]
                    pm = ps_w.tile([128, 512], f32, tag="ps_w",
                                   name=f"pm_{bi}_{wn}_{mo}_{q0}")
                    for ko in range(KO):
                        nc.tensor.matmul(
                            pm[:, :qw],
                            wb[wn][:, ko, mo * 128:(mo + 1) * 128],
                            XT[:, ko, q0:q0 + qw],
                            start=(ko == 0), stop=(ko == KO - 1))
                    if bias:
                        nc.vector.tensor_tensor(
                            dst[:, mo, q0:q0 + qw], pm[:, :qw],
                            bq_sb[:, mo:mo + 1].to_broadcast((128, qw)), ADD)
                    else:
                        nc.vector.tensor_copy(dst[:, mo, q0:q0 + qw], pm[:, :qw])

                def emit_rope_t(mo, which):
                    tgt = (QT, KT)[which]
                    src = tgt[:, mo, NPREF:T]
                    t1 = rpool.tile([128, PATCH], bf16, tag="rope1")
                    nc.vector.tensor_tensor(t1[:], src, cosT2[:], MUL)
                    t2 = rpool.tile([128, PATCH], bf16, tag="rope2")
                    for (o, sp) in ((0, 32), (32, 0), (64, 96), (96, 64)):
                        nc.vector.tensor_tensor(
                            t2[o:o + 32, :], tgt[sp:sp + 32, mo, NPREF:T],
                            sinT2sw[sp:sp + 32, :], MUL)
                    nc.vector.tensor_tensor(src, t1[:], t2[:], ADD)

                def emit_attn(h, pump=None):
                    ph = (h % 2) * 64
                    kq = h // 2
                    po = ps_o.tile([128, 1024], f32, tag="ps_o",
                                   name=f"po_{bi}_{h}")
                    for ji, (j0, jw) in enumerate(TOK_TILES):
                        if pump is not None:
                            pump()
                        first, last = ji == 0, ji == NJT - 1
                        pss = ps_s.tile([128, 1024], f32, tag="ps_s")
                        for qi, (q0, qw) in enumerate(QCHUNKS):
                            nc.tensor.matmul(
                                pss[:jw, q0:q0 + qw],
                                KT[ph:ph + 64, kq, j0:j0 + jw],
                                QT[ph:ph + 64, kq, q0:q0 + qw],
                                start=True, stop=True)
                        es = apool.tile([128, 1024], bf16, tag="expS")
                        nc.scalar.activation(es[:jw, :], pss[:jw, :],
                                             FP.Exp, scale=SCALE)
                        for qi, (q0, qw) in enumerate(QCHUNKS):
                            nc.tensor.matmul(
                                po[:HD + 1, q0:q0 + qw],
                                Vst[:jw, ji, h, :],
                                es[:jw, q0:q0 + qw],
                                start=first, stop=last)
                    for qi, (q0, qw) in enumerate(QCHUNKS):
                        rc = apool2.tile([1, 512], f32, tag="recip")
                        nc.vector.reciprocal(rc[0:1, :qw], po[64:65, q0:q0 + qw])
                        rb = apool2.tile([64, 512], f32, tag="recipB")
                        nc.gpsimd.partition_broadcast(rb[:, :qw], rc[0:1, :qw])
                        nc.vector.tensor_tensor(
                            AOT[ph:ph + 64, kq, q0:q0 + qw],
                            po[0:64, q0:q0 + qw], rb[:, :qw], MUL)

                def emit_tail():
                    # 5-query tail for all 16 heads, batched: S packed into one
                    # ps_s slot (heads 0..10 bank A, 11..15 bank B), two exps,
                    # AV accumulated per head into one ps_o slot.
                    qt0, qtw = QTAIL
                    pst = ps_s.tile([128, 1024], f32, tag="ps_s",
                                    name=f"pst_{bi}")
                    nc.vector.memset(pst[:], 0.0)

                    def tcol(h):
                        return (h * qtw * NJT if h <= 10
                                else 512 + (h - 11) * qtw * NJT)

                    for h in range(NH):
                        ph = (h % 2) * 64
                        kq = h // 2
                        for ji, (j0, jw) in enumerate(TOK_TILES):
                            nc.tensor.matmul(
                                pst[:jw,
                                    tcol(h) + ji * qtw: tcol(h) + (ji + 1) * qtw],
                                KT[ph:ph + 64, kq, j0:j0 + jw],
                                QT[ph:ph + 64, kq, qt0:qt0 + qtw],
                                start=True, stop=True)
                    est = apool.tile([128, 1024], bf16, tag="expS",
                                     name=f"est_{bi}")
                    nc.scalar.activation(est[:, 0:495], pst[:, 0:495],
                                         FP.Exp, scale=SCALE)
                    nc.scalar.activation(est[:, 512:737], pst[:, 512:737],
                                         FP.Exp, scale=SCALE)
                    pot = ps_o.tile([128, 1024], f32, tag="ps_o",
                                    name=f"pot_{bi}")
                    for h in range(NH):
                        for ji, (j0, jw) in enumerate(TOK_TILES):
                            nc.tensor.matmul(
                                pot[:HD + 1, h * qtw:(h + 1) * qtw],
                                Vst[:jw, ji, h, :],
                                est[0:jw,
                                    tcol(h) + ji * qtw: tcol(h) + (ji + 1) * qtw],
                                start=(ji == 0), stop=(ji == NJT - 1))
                    rc = apool2.tile([1, 512], f32, tag="recip")
                    nc.vector.reciprocal(rc[0:1, :NH * qtw],
                                         pot[64:65, :NH * qtw])
                    rb = apool2.tile([64, 512], f32, tag="recipB")
                    nc.gpsimd.partition_broadcast(rb[:, :NH * qtw],
                                                  rc[0:1, :NH * qtw])
                    for h in range(NH):
                        nc.vector.tensor_tensor(
                            AOT[(h % 2) * 64:(h % 2) * 64 + 64, h // 2,
                                qt0:qt0 + qtw],
                            pot[0:64, h * qtw:(h + 1) * qtw],
                            rb[:, h * qtw:(h + 1) * qtw], MUL)

                def emit_outproj_g(ti, nci):
                    t0, tw = TOK_TILES[ti]
                    n0, nw = NCHUNKS[nci]
                    pm = ps_w.tile([128, 512], f32, tag="ps_w",
                                   name=f"pmo_{bi}_{ti}_{n0}")
                    for ko in range(KO):
                        nc.tensor.matmul(
                            pm[:tw, :nw],
                            AOT[:, ko, t0:t0 + tw],
                            wb["wo"][:, ko, n0:n0 + nw],
                            start=(ko == 0), stop=(ko == KO - 1))
                    y = ypool.tile([128, 512], f32, tag="y")
                    nc.vector.tensor_tensor(y[:tw, :nw], pm[:tw, :nw],
                                            bo_bc[:tw, n0:n0 + nw], ADD)
                    nc.sync.dma_start(
                        out_d[tok0 + t0: tok0 + t0 + tw, n0:n0 + nw],
                        y[:tw, :nw])

                def emit_outproj(skip=()):
                    for ti in range(NJT):
                        for nci in range(len(NCHUNKS)):
                            if (ti, nci) not in skip:
                                emit_outproj_g(ti, nci)

                def emit_blocks(extra=None):
                    fills = []

                    def pump():
                        if fills:
                            fills.pop(0)()

                    for mo in range(KO):
                        if mo == 3:
                            fills.extend(
                                (lambda ti=ti: emit_vproj_t(1, ti))
                                for ti in range(NJT))
                        if mo < KO - 1:
                            fills.extend(
                                (lambda mo=mo, which=which, ci=ci:
                                 emit_qkproj_g(mo + 1, which, ci))
                                for which in range(2)
                                for ci in range(len(PROJ_CHUNKS)))
                            fills.append(lambda mo=mo: emit_rope_t(mo + 1, 0))
                            fills.append(lambda mo=mo: emit_rope_t(mo + 1, 1))
                        if extra and mo in extra:
                            fills.extend(extra[mo])
                        emit_attn(2 * mo, pump)
                        emit_attn(2 * mo + 1, pump)
                    while fills:
                        fills.pop(0)()

                def emit_head():
                    emit_vinit()
                    for ti in range(NJT):
                        emit_vproj_t(0, ti)
                    for which in range(2):
                        for ci in range(len(PROJ_CHUNKS)):
                            emit_qkproj_g(0, which, ci)
                    emit_rope_t(0, 0)
                    emit_rope_t(0, 1)

                return {
                    "head": emit_head, "blocks": emit_blocks,
                    "tail": emit_tail, "outproj": emit_outproj,
                    "outproj_g": emit_outproj_g,
                }

            it0 = make_item(0, XT0)
            it0["head"]()
            XT1 = ipool.tile([128, KO, T], bf16, tag="XT", name="XT_1")
            it0["blocks"](extra={7: [lambda: emit_xprep_full(1, XT1)]})
            it0["tail"]()
            it1 = make_item(1, XT1)
            it1["head"]()              # runs during item0 out-proj
            defer = [(ti, nci) for ti in range(5, NJT)
                     for nci in range(len(NCHUNKS))]
            it0["outproj"](skip=defer)
            dthunks = [(lambda ti=ti, nci=nci: it0["outproj_g"](ti, nci))
                       for (ti, nci) in defer]
            it1["blocks"](extra={6: dthunks[0:4], 7: dthunks[4:8]})
            it1["tail"]()
            it1["outproj"]()

    nc.compile()
    return nc


_NC_CACHE = []
_LAST_RESULT = []


def kernel(hidden_states, cos, sin, wq, bq, wk, wv, bv, wo, bo):
    from concourse.bass_utils import run_bass_kernel_spmd

    def _bf16(x):
        return np.ascontiguousarray(np.asarray(x).astype(ml_dtypes.bfloat16))

    def _f32(x):
        return np.ascontiguousarray(np.asarray(x, dtype=np.float32))

    hs_b = _bf16(hidden_states).reshape(B * T, H)
    shared = {
        "ident": np.eye(128, dtype=ml_dtypes.bfloat16),
        "cos": _f32(cos), "sin": _f32(sin),
        "wq": _bf16(wq), "wk": _bf16(wk), "wv": _bf16(wv), "wo": _bf16(wo),
        "bq": _f32(bq), "bv": _bf16(bv), "bo": _bf16(bo),
    }
    if not _NC_CACHE:
        _NC_CACHE.append(build())
    nc = _NC_CACHE[0]

    in_maps = []
    for c in range(NCORES):
        m = dict(shared)
        m["hs"] = np.ascontiguousarray(hs_b[c * TOK:(c + 1) * TOK].T)
        in_maps.append(m)

    try:
        res = run_bass_kernel_spmd(nc, in_maps, core_ids=list(range(NCORES)))
    except Exception:
        # transient NRT device errors (e.g. NRT_EXEC_UNIT_UNRECOVERABLE) have
        # been observed on this fabric; one retry usually succeeds
        time.sleep(2.0)
        res = run_bass_kernel_spmd(nc, in_maps, core_ids=list(range(NCORES)))
    _LAST_RESULT.clear()
    _LAST_RESULT.append(res)
    out = np.concatenate(
        [r["out"].reshape(BPC, T, H) for r in res.results], axis=0)
    return out

